# revision 17
# baseline (speedup 1.0000x reference)
"""Multi-head causal attention (B=4, S=2048, D=1024, H=16) on 8 trn2 cores.

Sharding: head-parallel. Core c owns heads 2c, 2c+1 (a 128-wide slice of the
qkv feature dim). Each core computes, for all 4 batches:
  QT/KT/VT = (x @ W^T)^T slices   [128 feats, 2048 tokens]  (feats on partitions)
  S^T      = K Q^T per head       [128 ktok, 512 q] blocks (causal rectangles)
  P^T      = exp(0.125 * S^T) * mask01  (multiplicative causal mask post-exp)
  O^T,Z    = [V_h | 1].T @ P^T    [65, 512] PSUM accumulated over ktok chunks
  otn      = O^T * (1/Z broadcast via K=1 matmul)
  partial  = otn.T @ wo_slice     [q 128, 1024]  -> DRAM
Host sums the 8 partial outputs (w_o row-parallel reduction) and adds b_o.

Matmul streams are float32r (full PE rate at N>=256; fp32 storage, reduced
mantissa multiply, fp32 PSUM accumulate).

fp32r matmuls self-load weights; walrus allows only ONE sync wait on the
fused LDW struct. So every SBUF operand of a matmul is produced on DVE
(DMA'd data goes through a DVE staging copy), causal masks are applied
multiplicatively on the exp output (keeps score-PSUM slot release ACT-only),
and a dummy PE transpose "observes" the vn DVE tick before AV matmuls.
check_mm_waits() verifies the <=1-wait invariant after scheduling.
"""

import sys

sys.path.insert(0, "/opt/trn_rl_repo")

import numpy as np

import concourse.bass as bass
import concourse.mybir as mybir
from concourse import tile
from concourse.bass_utils import run_bass_kernel_spmd

B, S, D, H, DH = 4, 2048, 1024, 16, 64
NCORES = 8
HPC = H // NCORES          # heads per core
F = HPC * DH               # per-core feature slice = 128
QR = 512                   # q-range tile
KC = 128                   # k-token chunk
NQR = S // QR              # 4
NKC_FULL = S // KC         # 16
NKD = D // 128             # 8 contraction chunks for projections
FP32 = mybir.dt.float32
FP32R = mybir.dt.float32r
EXP = mybir.ActivationFunctionType.Exp

_CACHE = {}


def build_program(stream_dt=FP32R):
    SD = stream_dt
    nc = bass.Bass("TRN2", debug=False)

    xt_d = nc.declare_dram_parameter("xt", [D, B * S], SD, isOutput=False)
    wq_d = nc.declare_dram_parameter("wq", [128, D], SD, isOutput=False)
    wk_d = nc.declare_dram_parameter("wk", [128, D], SD, isOutput=False)
    wv_d = nc.declare_dram_parameter("wv", [128, D], SD, isOutput=False)
    wo_d = nc.declare_dram_parameter("wo", [F, D], SD, isOutput=False)
    mask_d = nc.declare_dram_parameter("masks", [KC, 4 * QR], FP32, isOutput=False)
    id_d = nc.declare_dram_parameter("ident", [128, 128], SD, isOutput=False)
    out_d = nc.declare_dram_parameter("out", [B * S, D], FP32, isOutput=True)

    with tile.TileContext(nc) as tc:
        with (
            tc.tile_pool(name="static", bufs=1) as stat,
            tc.tile_pool(name="stage", bufs=4) as stagep,
            tc.tile_pool(name="perb", bufs=2) as perb,
            tc.tile_pool(name="xin", bufs=8) as xin,
            tc.tile_pool(name="pt", bufs=3) as ptp,
            tc.tile_pool(name="otn", bufs=2) as otnp,
            tc.tile_pool(name="obuf", bufs=3) as obufp,
            tc.tile_pool(name="small", bufs=4) as smallp,
            tc.tile_pool(name="ps_s", bufs=2, space="PSUM") as ps_s,
            tc.tile_pool(name="ps_ot", bufs=2, space="PSUM") as ps_ot,
            tc.tile_pool(name="ps_a", bufs=2, space="PSUM") as ps_a,
        ):
            # ---- static tiles: DMA -> stage, DVE copy -> target ----
            wq_sb = stat.tile([128, D], SD)
            wk_sb = stat.tile([128, D], SD)
            wv_sb = stat.tile([128, D], SD)
            wo_sb = stat.tile([F, D], SD)
            mask_sb = stat.tile([KC, 4 * QR], FP32)
            id_sb = stat.tile([128, 128], SD)
            ones_sb = stat.tile([1, 64], SD)
            for dram, dst in (
                (wq_d, wq_sb),
                (wk_d, wk_sb),
                (wv_d, wv_sb),
                (wo_d, wo_sb),
                (id_d, id_sb),
            ):
                stg = stagep.tile([128, D], SD, tag="stg")
                nc.sync.dma_start(stg[: dst.shape[0], : dst.shape[1]], dram[:])
                nc.vector.tensor_copy(dst[:], stg[: dst.shape[0], : dst.shape[1]])
            nc.sync.dma_start(mask_sb[:], mask_d[:])  # DVE-only consumer
            # ISA memset can't write fp32r; memset fp32 scratch, copy-convert
            onesrc = stat.tile([128, NKC_FULL * 130], FP32)
            nc.vector.memset(onesrc[:], 1.0)
            nc.vector.tensor_copy(ones_sb[:], onesrc[0:1, 0:64])

            for b in range(B):
                tok0 = b * S
                # ---- projections: QT/KT/VT [128 feats, S] ----
                qt = perb.tile([128, S], SD, tag="qt")
                kt = perb.tile([128, S], SD, tag="kt")
                vt = perb.tile([128, S], SD, tag="vt")
                vn = perb.tile([128, NKC_FULL * 130], SD, tag="vn")
                nc.vector.tensor_copy(vn[:], onesrc[:])
                for qr in range(NQR):
                    c0 = qr * QR
                    xts = []
                    for kd in range(NKD):
                        xstg = stagep.tile([128, QR], SD, tag="xstg")
                        nc.sync.dma_start(
                            xstg[:],
                            xt_d[kd * 128 : (kd + 1) * 128, tok0 + c0 : tok0 + c0 + QR],
                        )
                        xt_t = xin.tile([128, QR], SD)
                        nc.vector.tensor_copy(xt_t[:], xstg[:])
                        xts.append(xt_t)
                    for w_sb, dst in ((wq_sb, qt), (wk_sb, kt), (wv_sb, vt)):
                        ps = ps_a.tile([128, QR], FP32, tag="pa")
                        for kd in range(NKD):
                            nc.tensor.matmul(
                                ps[:],
                                w_sb[:, kd * 128 : (kd + 1) * 128],
                                xts[kd][:],
                                start=(kd == 0),
                                stop=(kd == NKD - 1),
                            )
                        nc.vector.tensor_copy(dst[:, c0 : c0 + QR], ps[:])

                # ---- V natural [ktok, feats] with ones cols, via PE transpose ----
                for tb in range(NKC_FULL):
                    for hh in range(HPC):
                        pst = ps_a.tile([128, 64], SD, tag="pa")
                        nc.tensor.transpose(
                            pst[:],
                            vt[hh * 64 : (hh + 1) * 64, tb * 128 : (tb + 1) * 128],
                            id_sb[hh * 64 : (hh + 1) * 64, hh * 64 : (hh + 1) * 64],
                        )
                        nc.vector.tensor_copy(
                            vn[:, tb * 130 + hh * 65 : tb * 130 + hh * 65 + 64], pst[:]
                        )
                # dummy transpose: PE observes vn's DVE tick before AV matmuls
                dmy = ps_a.tile([64, 64], SD, tag="pa")
                nc.tensor.transpose(
                    dmy[:],
                    vn[0:64, (NKC_FULL - 1) * 130 + 65 : (NKC_FULL - 1) * 130 + 129],
                    id_sb[0:64, 0:64],
                )

                # ---- attention per q-range ----
                for qr in range(NQR):
                    c0 = qr * QR
                    nkc = (qr + 1) * (QR // KC)
                    otp0 = ps_ot.tile([65, QR], FP32, tag="ot")
                    otp1 = ps_ot.tile([65, QR], FP32, tag="ot")
                    npair = nkc // 2
                    for t in range(npair):
                        kc0 = 2 * t
                        sp0 = ps_s.tile([128, 2 * QR], FP32, tag="s")
                        sp1 = ps_s.tile([128, 2 * QR], FP32, tag="s")
                        for j, kc in enumerate((kc0, kc0 + 1)):
                            kk = kc * KC
                            nc.tensor.matmul(
                                sp0[:, j * QR : (j + 1) * QR],
                                kt[0:64, kk : kk + KC],
                                qt[0:64, c0 : c0 + QR],
                                start=True, stop=True,
                                tile_position=(0, 0),
                            )
                            nc.tensor.matmul(
                                sp1[:, j * QR : (j + 1) * QR],
                                kt[64:128, kk : kk + KC],
                                qt[64:128, c0 : c0 + QR],
                                start=True, stop=True,
                                tile_position=(64, 0),
                            )
                        pt0 = ptp.tile([128, 2 * QR], SD, tag="pt")
                        pt1 = ptp.tile([128, 2 * QR], SD, tag="pt")
                        nc.scalar.activation(pt0[:], sp0[:], EXP, scale=0.125)
                        nc.scalar.activation(pt1[:], sp1[:], EXP, scale=0.125)
                        if kc0 >= qr * 4:  # diagonal pair -> multiplicative mask
                            o = kc0 - qr * 4
                            ms = mask_sb[:, o * QR : (o + 2) * QR]
                            nc.vector.tensor_mul(pt0[:], pt0[:], ms)
                            nc.vector.tensor_mul(pt1[:], pt1[:], ms)
                        for j, kc in enumerate((kc0, kc0 + 1)):
                            vb = kc * 130
                            nc.tensor.matmul(
                                otp0[:],
                                vn[:, vb : vb + 65],
                                pt0[:, j * QR : (j + 1) * QR],
                                start=(kc == 0), stop=(kc == nkc - 1),
                            )
                            nc.tensor.matmul(
                                otp1[:],
                                vn[:, vb + 65 : vb + 130],
                                pt1[:, j * QR : (j + 1) * QR],
                                start=(kc == 0), stop=(kc == nkc - 1),
                            )
                    # ---- normalize: otn = O^T * (1/Z) ----
                    rz0 = smallp.tile([1, QR], SD, tag="rz")
                    rz1 = smallp.tile([1, QR], SD, tag="rz")
                    with nc.allow_low_precision(
                        reason="fp32r is 4-byte fp32 storage; rounding only"
                    ):
                        nc.vector.reciprocal(rz0[:], otp0[64:65, :])
                        nc.vector.reciprocal(rz1[:], otp1[64:65, :])
                    rp0 = ps_a.tile([64, QR], FP32, tag="pa")
                    nc.tensor.matmul(rp0[:], ones_sb[:], rz0[:], start=True, stop=True)
                    rp1 = ps_a.tile([64, QR], FP32, tag="pa")
                    nc.tensor.matmul(rp1[:], ones_sb[:], rz1[:], start=True, stop=True)
                    rs0 = smallp.tile([64, QR], FP32, tag="rs")
                    rs1 = smallp.tile([64, QR], FP32, tag="rs")
                    nc.vector.tensor_copy(rs0[:], rp0[:])
                    nc.vector.tensor_copy(rs1[:], rp1[:])
                    otn = otnp.tile([128, QR], SD)
                    nc.vector.tensor_mul(otn[0:64, :], otp0[0:64, :], rs0[:])
                    nc.vector.tensor_mul(otn[64:128, :], otp1[0:64, :], rs1[:])
                    # ---- partial output projection ----
                    for qb in range(QR // 128):
                        ob = obufp.tile([128, D], FP32)
                        for half in range(2):
                            fp = ps_a.tile([128, 512], FP32, tag="pa")
                            nc.tensor.matmul(
                                fp[:],
                                otn[:, qb * 128 : (qb + 1) * 128],
                                wo_sb[:, half * 512 : (half + 1) * 512],
                                start=True, stop=True,
                            )
                            nc.vector.tensor_copy(
                                ob[:, half * 512 : (half + 1) * 512], fp[:]
                            )
                        row = tok0 + c0 + qb * 128
                        nc.sync.dma_start(out_d[row : row + 128, :], ob[:])
    _legalize_mm_waits(nc)
    return nc


def _legalize_mm_waits(nc):
    """TPB instructions have a single hw sync-wait slot and walrus does not
    auto-split (except for Drain). Move extra waits onto same-engine NoOps
    inserted right before the instruction."""
    import bass_rust

    skip = (mybir.InstAllEngineBarrier,)
    moved = 0
    for f in nc.m.functions:
        for bb in f.blocks:
            insts = bb.instructions
            i = 0
            while i < len(insts):
                inst = insts[i]
                if not isinstance(inst, skip) and not isinstance(
                    inst, mybir.InstNoOp
                ):
                    si = inst.sync_info
                    if si is not None and len(si.on_wait) > 1:
                        waits = list(si.on_wait)
                        for w in waits[:-1]:
                            nop = mybir.InstNoOp(
                                name=f"{inst.name}-wlg{moved}", ins=[], outs=[]
                            )
                            nop.engine = inst.engine
                            nop.sync_info = bass_rust.SyncInfo(
                                on_wait=[w], on_update=[]
                            )
                            insts.insert(i, nop)
                            i += 1
                            moved += 1
                        inst.sync_info = bass_rust.SyncInfo(
                            on_wait=[waits[-1]], on_update=list(si.on_update)
                        )
                i += 1
    return moved


def check_mm_waits(nc, limit=1):
    """fp32/fp32r matmuls fuse LDW+MM; walrus allows only `limit` sync waits."""
    bad = []
    for name, inst in nc.inst_map.items():
        if type(inst).__name__ == "InstMatmult":
            si = inst.sync_info
            if si is not None and len(si.on_wait) > limit:
                bad.append((name, [w.ant_name for w in si.on_wait]))
    return bad


def _pack_w(w_slice):
    """[128 outfeats, D] torch-Linear slice -> SBUF lhsT layout [128, D]:
    cols kd*128:(kd+1)*128 hold W^T[kd*128:(kd+1)*128, :]."""
    wt = np.ascontiguousarray(w_slice.T)  # [D, 128]
    return np.ascontiguousarray(
        wt.reshape(NKD, 128, 128).transpose(1, 0, 2).reshape(128, NKD * 128)
    )


def _masks():
    """Multiplicative causal masks: 4 diagonal offsets side by side."""
    k = np.arange(KC)[:, None]
    q = np.arange(QR)[None, :]
    cols = []
    for o in range(4):
        cols.append((o * KC + k <= q).astype(np.float32))
    return np.ascontiguousarray(np.concatenate(cols, axis=1))


def kernel(x, w_q, w_k, w_v, w_o, b_o):
    x = np.asarray(x, np.float32)
    w_q = np.asarray(w_q, np.float32)
    w_k = np.asarray(w_k, np.float32)
    w_v = np.asarray(w_v, np.float32)
    w_o = np.asarray(w_o, np.float32)
    b_o = np.asarray(b_o, np.float32)

    if "nc" not in _CACHE:
        _CACHE["nc"] = build_program()
    nc = _CACHE["nc"]

    xt = np.ascontiguousarray(x.reshape(B * S, D).T)
    masks = _masks()
    ident = np.eye(128, dtype=np.float32)
    in_maps = []
    for c in range(NCORES):
        cs, ce = c * F, (c + 1) * F
        in_maps.append(
            {
                "xt": xt,
                "wq": _pack_w(w_q[cs:ce]),
                "wk": _pack_w(w_k[cs:ce]),
                "wv": _pack_w(w_v[cs:ce]),
                "wo": np.ascontiguousarray(w_o[:, cs:ce].T),
                "masks": masks,
                "ident": ident,
            }
        )
    res = run_bass_kernel_spmd(nc, in_maps, list(range(NCORES)))
    acc = res.results[0]["out"].astype(np.float32)
    for i in range(1, NCORES):
        acc = acc + res.results[i]["out"]
    out = acc.reshape(B, S, D)
    if b_o.any():
        out = out + b_o
    return out


if __name__ == "__main__":
    nc = build_program()
    bad = check_mm_waits(nc)
    print(f"matmuls with >1 wait: {len(bad)}")
    for name, waits in bad[:20]:
        print(" ", name, waits)
